# revision 1
# baseline (speedup 1.0000x reference)
"""Trainium2 Bass kernel for nn_DecoderLayer (5-attention decoder layer + FFN).

Strategy: pure data-parallel over batch. B=16 across 8 cores -> 2 batch
elements per core, no collectives. Per core, activations are kept
feature-major (d on partitions, tokens on the free axis) so every matmul
chains without transposes; attention uses k-major scores and a token-major
numerator with the softmax denominator fused in as an extra ones-column of V.
LayerNorm gamma/beta and the 1/sqrt(dk) scale are folded into the projection
weights on the host; the causal self-attention skips fully-masked blocks.
Weights are bf16, accumulation fp32.
"""

import sys

sys.path.insert(0, "/opt/trn_rl_repo")

import numpy as np
import ml_dtypes

import concourse.bass as bass
import concourse.tile as tile
from concourse import bacc, mybir
from concourse.bass_utils import run_bass_kernel_spmd
from concourse.masks import make_identity
from contextlib import ExitStack

F32 = mybir.dt.float32
BF16 = mybir.dt.bfloat16
AF = mybir.ActivationFunctionType
ALU = mybir.AluOpType
BF = ml_dtypes.bfloat16

P = 128
D = 1024
KT = 8           # number of 128-partition tiles covering D
T = 1024         # tokens per core (2 batches x 512)
CH = 512         # free-dim chunk = one local batch worth of tokens
NCH = 2          # chunks (= local batches)
H = 16
DK = 64
DFF = 4096
NJ = DFF // P    # 32
N1 = 512
EPS = 1e-5
NEG = -1e9

_CACHE = {}


def _build(causal=True):
    nc = bacc.Bacc("TRN2", target_bir_lowering=False, debug=False, num_devices=8)

    xT = nc.dram_tensor("xT", [NCH, D, N1], F32, kind="ExternalInput")
    maskTd = nc.dram_tensor("maskTd", [NCH, 4, P, P], BF16, kind="ExternalInput")
    cptT = nc.dram_tensor("cptT", [NCH, D, 25], BF16, kind="ExternalInput")
    senT = nc.dram_tensor("senT", [NCH, D, 25], BF16, kind="ExternalInput")
    regT = nc.dram_tensor("regT", [NCH, D, 196], BF16, kind="ExternalInput")
    spaT = nc.dram_tensor("spaT", [NCH, D, 196], BF16, kind="ExternalInput")
    attW = nc.dram_tensor("attW", [5, 4, D, D], BF16, kind="ExternalInput")
    ffnW1 = nc.dram_tensor("ffnW1", [D, DFF], BF16, kind="ExternalInput")
    ffnW2 = nc.dram_tensor("ffnW2", [DFF, D], BF16, kind="ExternalInput")
    bqd = nc.dram_tensor("bq", [5, KT, P], F32, kind="ExternalInput")
    boed = nc.dram_tensor("boe", [5, KT, P], F32, kind="ExternalInput")
    b1d = nc.dram_tensor("b1", [NJ, P], F32, kind="ExternalInput")
    browd = nc.dram_tensor("brow", [1, 2, D], BF16, kind="ExternalInput")  # boe0, b2
    outT = nc.dram_tensor("outT", [NCH, D, N1], F32, kind="ExternalOutput")

    words = {"cpt": (cptT, 25), "sen": (senT, 25), "reg": (regT, 196), "spa": (spaT, 196)}

    with tile.TileContext(nc) as tc, ExitStack() as ctx:
        const = ctx.enter_context(tc.tile_pool(name="const", bufs=1))
        trunk_pool = ctx.enter_context(tc.tile_pool(name="trunk", bufs=1))
        spool = ctx.enter_context(tc.tile_pool(name="stats", bufs=1))
        tmp = ctx.enter_context(tc.tile_pool(name="tmps", bufs=2))
        wpool = ctx.enter_context(tc.tile_pool(name="weights", bufs=3))
        hpool = ctx.enter_context(tc.tile_pool(name="hq", bufs=1))
        apool = ctx.enter_context(tc.tile_pool(name="attn", bufs=1))
        psum = ctx.enter_context(tc.tile_pool(name="psum", bufs=1, space="PSUM"))
        prpool = ctx.enter_context(tc.tile_pool(name="probs", bufs=7))
        rcpool = ctx.enter_context(tc.tile_pool(name="recip", bufs=4))
        nmopool = ctx.enter_context(tc.tile_pool(name="nmo", bufs=4))
        mkpool = ctx.enter_context(tc.tile_pool(name="mk", bufs=1))
        w1pool = ctx.enter_context(tc.tile_pool(name="w1p", bufs=4))
        w2pool = ctx.enter_context(tc.tile_pool(name="w2p", bufs=3))
        outsb = ctx.enter_context(tc.tile_pool(name="outsb", bufs=2))

        _PBUFS = {"pA": 4, "pB": 2, "pC": 2}
        _pcnt = [0]

        def ptile(tag, shape=None, dtype=F32):
            # 8 PSUM banks shared by the whole kernel: 3+2+2+1 slots
            _pcnt[0] += 1
            return psum.tile(shape or [P, CH], dtype, name=f"{tag}_{_pcnt[0]}",
                             tag=tag, bufs=_PBUFS[tag])

        def dma(out, in_):
            nc.sync.dma_start(out=out, in_=in_)

        ident = const.tile([P, P], BF16, name="ident", tag="ident")
        make_identity(nc, ident)
        ones_col = const.tile([P, 1], BF16, name="ones_col", tag="ones_col")
        nc.vector.memset(ones_col, 1.0)
        ones_row = const.tile([1, P], BF16, name="ones_row", tag="ones_row")
        nc.vector.memset(ones_row, 1.0)
        ones_cn = const.tile([1, CH], BF16, name="ones_cn", tag="ones_cn")
        nc.vector.memset(ones_cn, 1.0)
        eps_sb = const.tile([P, 1], F32, name="eps_sb", tag="eps_sb")
        nc.vector.memset(eps_sb, EPS)
        brow = const.tile([1, 2, D], BF16, name="brow", tag="brow")
        dma(brow[:], browd.ap())

        def load_bias(dram, row, ntiles, tag):
            t = const.tile([P, ntiles], F32, tag=tag)
            src = dram.ap()[row] if row is not None else dram.ap()
            dma(t[:, :], src.rearrange("j p -> p j"))
            return t

        bq_sb = [load_bias(bqd, i, KT, f"bq{i}") for i in range(5)]
        boe_sb = [load_bias(boed, i, KT, f"boe{i}") for i in range(1, 5)]
        boe_sb = [None] + boe_sb
        b1_sb = load_bias(b1d, None, NJ, "b1")

        # trunk = captions^T, f32, updated in place through the layer
        trunk = []
        for k in range(KT):
            t = trunk_pool.tile([P, T], F32, name=f"c{k}", tag=f"c{k}")
            dma(t.rearrange("p (b n) -> p b n", b=NCH),
                xT.ap()[:, k * P:(k + 1) * P, :].rearrange("b p n -> p b n"))
            trunk.append(t)

        # ---------- layernorm (feature-major); gamma/beta folded into weights ----------
        def ln_stats():
            """Returns (u_sb, v_sb): [128, T] f32 tiles with rstd and
            -mean*rstd broadcast along partitions: (x*u + v) is normalized."""
            u_sb = spool.tile([P, T], BF16, name="u_sb", tag="u_sb")
            v_sb = spool.tile([P, T], BF16, name="v_sb", tag="v_sb")
            if True:
                for ch in range(NCH):
                    cs = slice(ch * CH, (ch + 1) * CH)
                    s1 = ptile("pC", [1, CH])
                    s2 = ptile("pC", [1, CH])
                    for k in range(KT):
                        xc = tmp.tile([P, CH], BF16, name="xc", tag="xc")
                        nc.vector.tensor_copy(xc[:], trunk[k][:, cs])
                        sq = tmp.tile([P, CH], BF16, name="sq", tag="sq")
                        nc.vector.tensor_mul(sq[:], xc[:], xc[:])
                        nc.tensor.matmul(s1[:], lhsT=ones_col[:], rhs=xc[:],
                                         start=(k == 0), stop=(k == KT - 1))
                        nc.tensor.matmul(s2[:], lhsT=ones_col[:], rhs=sq[:],
                                         start=(k == 0), stop=(k == KT - 1))
                    # rs = 1/sqrt((s2 - s1^2/D)/D + eps); u = rs; v = -(s1/D)*rs
                    s1c = tmp.tile([1, CH], F32, name="rs1", tag="rA", bufs=2)
                    nc.vector.tensor_copy(s1c[:], s1[:])
                    q = tmp.tile([1, CH], F32, name="rq", tag="rB", bufs=4)
                    nc.vector.tensor_mul(q[:], s1c[:], s1c[:])
                    v2 = tmp.tile([1, CH], F32, name="rv2", tag="rB", bufs=4)
                    nc.vector.tensor_scalar(out=v2[:], in0=q[:], scalar1=-1.0 / D,
                                            scalar2=None, op0=ALU.mult)
                    nc.vector.tensor_add(v2[:], v2[:], s2[:])
                    sd = tmp.tile([1, CH], F32, name="rsd", tag="rB", bufs=4)
                    nc.scalar.activation(sd[:], v2[:], AF.Sqrt, bias=eps_sb[0:1, :], scale=1.0 / D)
                    rs = tmp.tile([1, CH], F32, name="rrs", tag="rB", bufs=4)
                    nc.vector.reciprocal(rs[:], sd[:])
                    u_bf = tmp.tile([1, CH], BF16, name="ru", tag="rC", bufs=2)
                    nc.vector.tensor_copy(u_bf[:], rs[:])
                    w = tmp.tile([1, CH], F32, name="rw", tag="rB", bufs=4)
                    nc.vector.tensor_mul(w[:], s1c[:], rs[:])
                    v_bf = tmp.tile([1, CH], BF16, name="rvb", tag="rC", bufs=2)
                    nc.vector.tensor_scalar(out=v_bf[:], in0=w[:], scalar1=-1.0 / D,
                                            scalar2=None, op0=ALU.mult)
                    ub = ptile("pB")
                    nc.tensor.matmul(ub[:], lhsT=ones_row[:], rhs=u_bf[:], start=True, stop=True)
                    nc.vector.tensor_copy(u_sb[:, cs], ub[:])
                    vb = ptile("pB")
                    nc.tensor.matmul(vb[:], lhsT=ones_row[:], rhs=v_bf[:], start=True, stop=True)
                    nc.vector.tensor_copy(v_sb[:, cs], vb[:])
            return u_sb, v_sb

        def ln_nrm(u_sb, v_sb):
            """nrm[k] = trunk[k]*u + v -> bf16 tiles (tag h{k})."""
            hs = []
            for k in range(KT):
                h = hpool.tile([P, T], BF16, name=f"h{k}", tag=f"h{k}")
                for ch in range(NCH):
                    cs = slice(ch * CH, (ch + 1) * CH)
                    t1 = tmp.tile([P, CH], F32, name="lnt", tag="lnt")
                    nc.vector.tensor_mul(t1[:], trunk[k][:, cs], u_sb[:, cs])
                    nc.vector.tensor_add(h[:, cs], t1[:], v_sb[:, cs])
                hs.append(h)
            return hs

        # ---------- weight streaming: quarter-matrices [128, KT, 256] ----------
        QW = 256

        def load_attw_q(ai, j, qtr):
            t = wpool.tile([P, KT, QW], BF16, name="W", tag="W")
            dma(t[:], attW.ap()[ai, j, :, qtr * QW:(qtr + 1) * QW]
                .rearrange("(k p) c -> p k c", p=P))
            return t

        def proj_featmajor(ai, j, h_bf, out_tiles, bias_sb, act_func):
            """out^T[m][:, ch] = ((X W)^T + b)[m-tile, ch] for all m, ch."""
            for qtr in range(4):
                w = load_attw_q(ai, j, qtr)
                for mh in range(2):
                    m = qtr * 2 + mh
                    for ch in range(NCH):
                        ps = ptile("pA")
                        for k in range(KT):
                            nc.tensor.matmul(ps[:], lhsT=w[:, k, mh * P:(mh + 1) * P],
                                             rhs=h_bf[k][:, ch * CH:(ch + 1) * CH],
                                             start=(k == 0), stop=(k == KT - 1))
                        if bias_sb is not None:
                            nc.scalar.activation(out_tiles[m][:, ch * CH:(ch + 1) * CH],
                                                 ps[:], act_func, bias=bias_sb[:, m:m + 1])
                        else:
                            nc.scalar.copy(out_tiles[m][:, ch * CH:(ch + 1) * CH], ps[:])

        QT = [hpool.tile([P, T], BF16, name=f"q{m}", tag=f"q{m}") for m in range(KT)]
        AT = [apool.tile([P, T], BF16, name=f"aT{m}", tag=f"aT{m}") for m in range(KT)]

        def attn_core_pair(b, hp, probs2, plens, qoffs, VW):
            """numerators + normalize for heads (2hp, 2hp+1), one shared
            transpose of the [128, 128] pair block into AT[hp].

            probs2[e][i] holds q columns starting at global offset qoffs[i]."""
            for qt in range(4):
                nmo2 = nmopool.tile([P, 2 * DK], BF16, name="nmo2", tag="nmo2")
                for e in range(2):
                    h = 2 * hp + e
                    nm = ptile("pA", [P, 65])
                    kts = [i for i in range(len(probs2[e])) if qoffs[i] <= qt * P]
                    for n, i in enumerate(kts):
                        sz = plens[i]
                        c0 = qt * P - qoffs[i]
                        nc.tensor.matmul(nm[:], lhsT=probs2[e][i][:sz, c0:c0 + P],
                                         rhs=VW[i][:sz, 65 * h:65 * h + 65],
                                         start=(n == 0), stop=(n == len(kts) - 1))
                    rc = rcpool.tile([P, 1], F32, name="rc", tag="rc")
                    nc.vector.reciprocal(rc[:], nm[:, 64:65])
                    nc.any.tensor_scalar(out=nmo2[:, e * DK:(e + 1) * DK], in0=nm[:, 0:DK],
                                         scalar1=rc[:], scalar2=None, op0=ALU.mult)
                tp = ptile("pB", [P, P], BF16)
                nc.tensor.transpose(tp[:], nmo2[:], ident[:])
                nc.any.tensor_copy(
                    AT[hp][:, b * CH + qt * P:b * CH + (qt + 1) * P], tp[:])

        def o_proj(ai, residual, dst):
            """residual: trunk += Wo^T@AT + boe (bias injected via K=1 matmul);
            else dst[m] = Wo^T@AT + boe via activation bias."""
            if True:
                for qtr in range(4):
                    w = load_attw_q(ai, 3, qtr)
                    for mh in range(2):
                        m = qtr * 2 + mh
                        for ch in range(NCH):
                            cs = slice(ch * CH, (ch + 1) * CH)
                            ps = ptile("pA")
                            if residual:
                                nc.tensor.matmul(ps[:], lhsT=brow[0:1, 0, m * P:(m + 1) * P],
                                                 rhs=ones_cn[:], start=True, stop=False)
                            for k in range(KT):
                                nc.tensor.matmul(ps[:], lhsT=w[:, k, mh * P:(mh + 1) * P],
                                                 rhs=AT[k][:, cs],
                                                 start=False if residual else (k == 0),
                                                 stop=(k == KT - 1))
                            if residual:
                                nc.vector.tensor_add(trunk[m][:, cs], trunk[m][:, cs], ps[:])
                            else:
                                nc.scalar.activation(dst[m][:, cs], ps[:], AF.Identity,
                                                     bias=boe_sb[ai][:, m:m + 1])

        # ==================== LN0 + self-attention ====================
        u0, v0 = ln_stats()
        h0 = ln_nrm(u0, v0)

        KTl = [apool.tile([P, T], BF16, name=f"kt{m}", tag=f"kt{m}") for m in range(KT)]
        VA = [apool.tile([P, H * 65], BF16, name=f"va{g}", tag=f"va{g}") for g in range(KT)]
        if True:
            proj_featmajor(0, 0, h0, QT, bq_sb[0], AF.Identity)
            proj_featmajor(0, 1, h0, KTl, None, None)
            for g in range(KT):
                nc.vector.memset(VA[g].rearrange("p (h c) -> p h c", c=65)[:, :, 64:65], 1.0)
            for qtr in range(4):
                wv = load_attw_q(0, 2, qtr)
                for g in range(KT):
                    ps = ptile("pB", [P, QW])
                    for k in range(KT):
                        nc.tensor.matmul(ps[:], lhsT=h0[k][:, g * P:(g + 1) * P],
                                         rhs=wv[:, k, :],
                                         start=(k == 0), stop=(k == KT - 1))
                    dst = VA[g].rearrange("p (h c) -> p h c", c=65)[:, qtr * 4:(qtr + 1) * 4, 0:64]
                    nc.any.tensor_copy(dst, ps.rearrange("p (h c) -> p h c", c=64))

        if True:
            for b in range(NCH):
                mkt = mkpool.tile([P, 4, P], BF16, name="mkt", tag="mkt")
                dma(mkt[:], maskTd.ap()[b].rearrange("t p c -> p t c"))
                mk = [mkt[:, kt] for kt in range(4)]
                for hp in range(H // 2):
                    mt = hp
                    probs2 = []
                    for e in range(2):
                        po = e * 64
                        probs = []
                        for kt in range(4):
                            nq = CH - kt * P
                            q0 = kt * P
                            ps = ptile("pC")
                            nc.tensor.matmul(
                                ps[:, :nq],
                                lhsT=KTl[mt][po:po + 64, b * CH + kt * P:b * CH + (kt + 1) * P],
                                rhs=QT[mt][po:po + 64, b * CH + q0:(b + 1) * CH],
                                start=True, stop=False)
                            # additive mask via PE: ps[:,0:P] += mkq.T @ I
                            nc.tensor.matmul(ps[:, 0:P], lhsT=mk[kt][:], rhs=ident[:],
                                             start=False, stop=True,
                                             skip_group_check=True)
                            pr = prpool.tile([P, CH], BF16, name="pr", tag="pr")
                            nc.scalar.activation(pr[:, :nq], ps[:, :nq], AF.Exp)
                            probs.append(pr)
                        probs2.append(probs)
                    qoffs = [kt * P for kt in range(4)]
                    attn_core_pair(b, hp, probs2, [P] * 4, qoffs, VA[4 * b:4 * b + 4])
        o_proj(0, True, None)

        # ============ shared LN stats + normalized c; cross attentions ============
        uc, vc = ln_stats()
        nrm_c = ln_nrm(uc, vc)

        def cross_attention(ai, name, dst):
            w_dram, L = words[name]
            nkt = (L + P - 1) // P
            lsz = [min(P, L - kt * P) for kt in range(nkt)]
            wt_all = spool.tile([P, KT, NCH, L], BF16, name="wt_all", tag="wt_all")
            for b in range(NCH):
                dma(wt_all[:, :, b, :], w_dram.ap()[b].rearrange("(k p) l -> p k l", p=P))
            wtl = [wt_all[:, k] for k in range(KT)]
            KW = [spool.tile([P, NCH, L], BF16, name=f"kw{m}", tag=f"kw{m}") for m in range(KT)]
            VW = [spool.tile([P, H * 65], BF16, name=f"vaw{b}_{kt}", tag=f"vaw{b}_{kt}")
                  for b in range(NCH) for kt in range(nkt)]
            if True:
                proj_featmajor(ai, 0, nrm_c, QT, bq_sb[ai], AF.Identity)
                for qtr in range(4):
                    wk = load_attw_q(ai, 1, qtr)
                    for mh in range(2):
                        m = qtr * 2 + mh
                        for b in range(NCH):
                            ps = ptile("pC", [P, L])
                            for k in range(KT):
                                nc.tensor.matmul(ps[:], lhsT=wk[:, k, mh * P:(mh + 1) * P],
                                                 rhs=wtl[k][:, b, :],
                                                 start=(k == 0), stop=(k == KT - 1))
                            nc.any.tensor_copy(KW[m][:, b, :], ps[:])
                for va in VW:
                    nc.vector.memset(va.rearrange("p (h c) -> p h c", c=65)[:, :, 64:65], 1.0)
                for qtr in range(4):
                    wv = load_attw_q(ai, 2, qtr)
                    for b in range(NCH):
                        for kt in range(nkt):
                            sz = lsz[kt]
                            ps = ptile("pB", [P, QW])
                            for k in range(KT):
                                nc.tensor.matmul(ps[:sz, :],
                                                 lhsT=wtl[k][:, b, kt * P:kt * P + sz],
                                                 rhs=wv[:, k, :],
                                                 start=(k == 0), stop=(k == KT - 1))
                            va = VW[b * nkt + kt]
                            dstv = va.rearrange("p (h c) -> p h c", c=65)[:sz, qtr * 4:(qtr + 1) * 4, 0:64]
                            nc.any.tensor_copy(dstv, ps[:sz].rearrange("p (h c) -> p h c", c=64))
            if True:
                for b in range(NCH):
                    for hp in range(H // 2):
                        mt = hp
                        probs2 = []
                        for e in range(2):
                            po = e * 64
                            probs = []
                            for kt in range(nkt):
                                sz = lsz[kt]
                                ps = ptile("pC")
                                nc.tensor.matmul(ps[:sz, :],
                                                 lhsT=KW[mt][po:po + 64, b, kt * P:kt * P + sz],
                                                 rhs=QT[mt][po:po + 64, b * CH:(b + 1) * CH],
                                                 start=True, stop=True)
                                pr = prpool.tile([P, CH], BF16, name="pr", tag="pr")
                                nc.scalar.activation(pr[:sz, :], ps[:sz, :], AF.Exp)
                                probs.append(pr)
                            probs2.append(probs)
                        attn_core_pair(b, hp, probs2, lsz, [0] * nkt, VW[b * nkt:(b + 1) * nkt])
            o_proj(ai, False, dst)

        def fuse_gate(x1, x2):
            """trunk += 0.5 * softmax-gated combination of x1, x2 (vs pre-gate c).

            The gate-score reads of trunk are emitted before the in-place
            trunk updates below; Tile orders them correctly."""
            if True:
                for ch in range(NCH):
                    cs = slice(ch * CH, (ch + 1) * CH)
                    srow = []
                    for xi in (x1, x2):
                        s = ptile("pB", [1, CH])
                        for k in range(KT):
                            cbf = tmp.tile([P, CH], BF16, name="gcb", tag="gcb")
                            nc.vector.tensor_copy(cbf[:], trunk[k][:, cs])
                            nc.vector.tensor_mul(cbf[:], xi[k][:, cs], cbf[:])
                            nc.tensor.matmul(s[:], lhsT=ones_col[:], rhs=cbf[:],
                                             start=(k == 0), stop=(k == KT - 1))
                        srow.append(s)
                    e0 = tmp.tile([1, CH], F32, name="ge0", tag="rA", bufs=2)
                    nc.scalar.activation(e0[:], srow[0][:], AF.Exp)
                    e1 = tmp.tile([1, CH], F32, name="ge1", tag="rA", bufs=2)
                    nc.scalar.activation(e1[:], srow[1][:], AF.Exp)
                    ss = tmp.tile([1, CH], F32, name="gss", tag="rB", bufs=4)
                    nc.vector.tensor_add(ss[:], e0[:], e1[:])
                    r = tmp.tile([1, CH], F32, name="gr", tag="rB", bufs=4)
                    nc.vector.reciprocal(r[:], ss[:])
                    r2 = tmp.tile([1, CH], F32, name="gr2", tag="rB", bufs=4)
                    nc.vector.tensor_scalar(out=r2[:], in0=r[:], scalar1=0.5,
                                            scalar2=None, op0=ALU.mult)
                    w0 = tmp.tile([1, CH], BF16, name="gw0", tag="rC", bufs=2)
                    nc.vector.tensor_mul(w0[:], e0[:], r2[:])
                    w1 = tmp.tile([1, CH], BF16, name="gw1", tag="rC", bufs=2)
                    nc.vector.tensor_mul(w1[:], e1[:], r2[:])
                    w0b = ptile("pC")
                    nc.tensor.matmul(w0b[:], lhsT=ones_row[:], rhs=w0[:], start=True, stop=True)
                    w1b = ptile("pC")
                    nc.tensor.matmul(w1b[:], lhsT=ones_row[:], rhs=w1[:], start=True, stop=True)
                    for k in range(KT):
                        t1 = tmp.tile([P, CH], BF16, name="gt1", tag="gt1")
                        nc.vector.tensor_mul(t1[:], x1[k][:, cs], w0b[:])
                        nc.vector.tensor_add(trunk[k][:, cs], trunk[k][:, cs], t1[:])
                        t2 = tmp.tile([P, CH], BF16, name="gt2", tag="gt1")
                        nc.vector.tensor_mul(t2[:], x2[k][:, cs], w1b[:])
                        nc.vector.tensor_add(trunk[k][:, cs], trunk[k][:, cs], t2[:])

        cptO = [apool.tile([P, T], BF16, name=f"cptO{m}", tag=f"kt{m}") for m in range(KT)]
        senO = [apool.tile([P, T], BF16, name=f"senO{g}", tag=f"va{g}") for g in range(KT)]
        cross_attention(1, "cpt", cptO)
        cross_attention(2, "sen", senO)
        fuse_gate(cptO, senO)
        regO = [apool.tile([P, T], BF16, name=f"regO{m}", tag=f"kt{m}") for m in range(KT)]
        spaO = [apool.tile([P, T], BF16, name=f"spaO{g}", tag=f"va{g}") for g in range(KT)]
        cross_attention(3, "reg", regO)
        cross_attention(4, "spa", spaO)
        fuse_gate(regO, spaO)

        # ==================== LN5 + FFN ====================
        u5, v5 = ln_stats()
        h5 = ln_nrm(u5, v5)

        def mid_tag(j):
            if j < KT:
                return f"kt{j}"
            if j < 2 * KT:
                return f"va{j - KT}"
            if j < 3 * KT:
                return f"aT{j - 2 * KT}"
            return f"q{j - 3 * KT}"

        mid = [apool.tile([P, T], BF16, name=f"mid{j}", tag=mid_tag(j)) if j < 3 * KT
               else hpool.tile([P, T], BF16, name=f"mid{j}", tag=mid_tag(j)) for j in range(NJ)]
        if True:
            for j in range(NJ):
                w1t = w1pool.tile([P, KT, P], BF16, name="w1j", tag="w1j")
                dma(w1t[:], ffnW1.ap()[:, j * P:(j + 1) * P]
                    .rearrange("(k p) c -> p k c", p=P))
                for ch in range(NCH):
                    ps = ptile("pA")
                    for k in range(KT):
                        nc.tensor.matmul(ps[:], lhsT=w1t[:, k, :],
                                         rhs=h5[k][:, ch * CH:(ch + 1) * CH],
                                         start=(k == 0), stop=(k == KT - 1))
                    nc.scalar.activation(mid[j][:, ch * CH:(ch + 1) * CH], ps[:],
                                         AF.Relu, bias=b1_sb[:, j:j + 1])
        if True:
            for ch in range(NCH):
                cs = slice(ch * CH, (ch + 1) * CH)
                _lets = ["pA", "pA", "pA", "pA", "pB", "pB", "pC", "pC"]
                pss = [ptile(_lets[m]) for m in range(KT)]
                for m in range(KT):
                    nc.tensor.matmul(pss[m][:], lhsT=brow[0:1, 1, m * P:(m + 1) * P],
                                     rhs=ones_cn[:], start=True, stop=False)
                for j in range(NJ):
                    w2t = w2pool.tile([P, D], BF16, name="w2j", tag="w2j")
                    dma(w2t[:], ffnW2.ap()[j * P:(j + 1) * P, :])
                    for m in range(KT):
                        nc.tensor.matmul(pss[m][:], lhsT=w2t[:, m * P:(m + 1) * P],
                                         rhs=mid[j][:, cs],
                                         start=False, stop=(j == NJ - 1))
                for m in range(KT):
                    ot = outsb.tile([P, CH], F32, name="ot", tag="ot")
                    nc.vector.tensor_add(ot[:], trunk[m][:, cs], pss[m][:])
                    dma(outT.ap()[ch, m * P:(m + 1) * P, :], ot[:])

    nc.compile()
    return nc


def _check_causal(seq_masks):
    """The block-sparse self-attention path needs: below-diagonal k-blocks
    fully attended, above-diagonal fully masked (any within-diagonal-block
    pattern is handled exactly)."""
    m = np.asarray(seq_masks)
    for b in range(m.shape[0]):
        for qt in range(4):
            for kt in range(4):
                blk = m[b, qt * P:(qt + 1) * P, kt * P:(kt + 1) * P]
                if kt < qt and not (blk == 1).all():
                    return False
                if kt > qt and not (blk == 0).all():
                    return False
    return True


def _host_prep(inputs):
    captions = np.asarray(inputs["captions"], np.float32)
    seq_masks = np.asarray(inputs["seq_masks"])
    att_W = np.asarray(inputs["att_W"], np.float32)
    att_b = np.asarray(inputs["att_b"], np.float32)
    ln_g = np.asarray(inputs["ln_g"], np.float32)
    ln_b = np.asarray(inputs["ln_b"], np.float32)
    ffn_W1 = np.asarray(inputs["ffn_W1"], np.float32)
    ffn_W2 = np.asarray(inputs["ffn_W2"], np.float32)
    ffn_b1 = np.asarray(inputs["ffn_b1"], np.float32)
    ffn_b2 = np.asarray(inputs["ffn_b2"], np.float32)

    # Fold LN gamma/beta of the query-side layernorm and 1/sqrt(dk) into the
    # projections. Query-LN site for attention i: site 0 for i=0, site i for
    # cross. K bias drops (softmax shift invariance); V bias folds through O.
    Wq = np.empty_like(att_W[:, 0])
    Wk = att_W[:, 1].copy()
    Wv = att_W[:, 2].copy()
    Wo = att_W[:, 3]
    bq = np.empty_like(att_b[:, 0])
    boe = np.empty_like(att_b[:, 3])
    for i in range(5):
        s = 0 if i == 0 else i
        g, b = ln_g[s], ln_b[s]
        Wq[i] = 0.125 * (g[:, None] * att_W[i, 0])
        bq[i] = 0.125 * (b @ att_W[i, 0] + att_b[i, 0])
        if i == 0:
            # self-attention: K and V also consume the normalized input
            Wk[0] = g[:, None] * att_W[0, 1]
            Wv[0] = g[:, None] * att_W[0, 2]
            bV = b @ att_W[0, 2] + att_b[0, 2]
        else:
            bV = att_b[i, 2]
        boe[i] = bV @ att_W[i, 3] + att_b[i, 3]
    attW = np.stack([Wq, Wk, Wv, Wo], axis=1)
    g5, b5 = ln_g[5], ln_b[5]
    W1 = g5[:, None] * ffn_W1
    b1 = b5 @ ffn_W1 + ffn_b1
    brow = np.stack([boe[0], ffn_b2])[None].astype(BF)

    xT = np.ascontiguousarray(captions.transpose(0, 2, 1))

    def wT(name):
        a = np.asarray(inputs[name], np.float32)
        return np.ascontiguousarray(a.transpose(0, 2, 1)).astype(BF)

    prep = dict(
        xT=xT,
        cptT=wT("cpt_words"), senT=wT("senti_words"),
        regT=wT("region_feats"), spaT=wT("spatial_feats"),
        attW=attW.astype(BF),
        ffnW1=W1.astype(BF),
        ffnW2=ffn_W2.astype(BF),
        bq=np.ascontiguousarray(bq.reshape(5, KT, P)),
        boe=np.ascontiguousarray(boe.reshape(5, KT, P)),
        b1=np.ascontiguousarray(b1.reshape(NJ, P)),
        brow=brow,
    )
    mTd = np.zeros((seq_masks.shape[0], 4, P, P), np.float32)
    for kt in range(4):
        blk = seq_masks[:, kt * P:(kt + 1) * P, kt * P:(kt + 1) * P]  # [B, q, k]
        mTd[:, kt] = np.where(blk == 0, np.float32(NEG), 0.0)  # q-major lhsT
    prep["maskTd"] = mTd.astype(BF)
    return prep


def _numpy_reference(inputs):
    """Fallback for masks outside the block-causal structure the device
    kernel assumes. Bit-accurate fp32 numpy implementation."""
    f = lambda k: np.asarray(inputs[k], np.float32)
    att_W, att_b = f("att_W"), f("att_b")
    ln_g, ln_b = f("ln_g"), f("ln_b")
    mask = np.asarray(inputs["seq_masks"])

    def ln(x, g, b):
        m = x.mean(-1, keepdims=True)
        v = ((x - m) ** 2).mean(-1, keepdims=True)
        return (x - m) / np.sqrt(v + EPS) * g + b

    def mha(q_in, k_in, v_in, W, b, msk=None):
        B_, N = q_in.shape[0], q_in.shape[1]
        def proj(x, i):
            y = x @ W[i] + b[i]
            return y.reshape(x.shape[0], -1, H, DK).transpose(0, 2, 1, 3)
        q, k, v = proj(q_in, 0), proj(k_in, 1), proj(v_in, 2)
        s = (q @ k.transpose(0, 1, 3, 2)) / np.sqrt(DK)
        if msk is not None:
            s = np.where(msk[:, None] == 0, -np.inf, s)
        s = s - s.max(-1, keepdims=True)
        a = np.exp(s)
        a /= a.sum(-1, keepdims=True)
        x = (a @ v).transpose(0, 2, 1, 3).reshape(B_, N, H * DK)
        return x @ W[3] + b[3]

    def gate(x, f1, f2):
        s = np.stack([(f1 * x).sum(-1), (f2 * x).sum(-1)], -1)
        s = s - s.max(-1, keepdims=True)
        w = np.exp(s)
        w /= w.sum(-1, keepdims=True)
        return w[..., 0:1] * f1 + w[..., 1:2] * f2

    c = f("captions")
    h = ln(c, ln_g[0], ln_b[0])
    c = c + mha(h, h, h, att_W[0], att_b[0], mask)
    cpt = mha(ln(c, ln_g[1], ln_b[1]), f("cpt_words"), f("cpt_words"), att_W[1], att_b[1])
    sen = mha(ln(c, ln_g[2], ln_b[2]), f("senti_words"), f("senti_words"), att_W[2], att_b[2])
    sem = gate(c, cpt, sen)
    reg = mha(ln(c, ln_g[3], ln_b[3]), f("region_feats"), f("region_feats"), att_W[3], att_b[3])
    spa = mha(ln(c, ln_g[4], ln_b[4]), f("spatial_feats"), f("spatial_feats"), att_W[4], att_b[4])
    vis = gate(c, reg, spa)
    fuse = c + (sem + vis) * 0.5
    hh = ln(fuse, ln_g[5], ln_b[5])
    return fuse + np.maximum(hh @ f("ffn_W1") + f("ffn_b1"), 0) @ f("ffn_W2") + f("ffn_b2")


def kernel(**inputs) -> np.ndarray:
    if not _check_causal(inputs["seq_masks"]):
        return _numpy_reference(inputs).astype(np.float32)
    if "nc" not in _CACHE:
        _CACHE["nc"] = _build(True)
    nc = _CACHE["nc"]
    prep = _host_prep(inputs)
    B = inputs["captions"].shape[0]
    n_cores = 8
    bl = B // n_cores
    shared_keys = ("attW", "ffnW1", "ffnW2", "bq", "boe", "b1", "brow")
    per_core_keys = ["xT", "cptT", "senT", "regT", "spaT", "maskTd"]
    in_maps = []
    for i in range(n_cores):
        s = slice(i * bl, (i + 1) * bl)
        m = {k: prep[k] for k in shared_keys}
        for k in per_core_keys:
            m[k] = prep[k][s]
        in_maps.append(m)
    res = run_bass_kernel_spmd(nc, in_maps, list(range(n_cores)))
    out = np.empty((B, N1, D), np.float32)
    for i in range(n_cores):
        out[i * bl:(i + 1) * bl] = res.results[i]["outT"].transpose(0, 2, 1)
    return out



# revision 3
# speedup vs baseline: 1.2575x; 1.2575x over previous
"""Trainium2 Bass kernel for nn_DecoderLayer (5-attention decoder layer + FFN).

Strategy: pure data-parallel over batch. B=16 across 8 cores -> 2 batch
elements per core, no collectives. Per core, activations are kept
feature-major (d on partitions, tokens on the free axis) so every matmul
chains without transposes; attention uses k-major scores and a token-major
numerator with the softmax denominator fused in as an extra ones-column of V.
LayerNorm gamma/beta and the 1/sqrt(dk) scale are folded into the projection
weights on the host; the causal self-attention skips fully-masked blocks.

All attention projections (Q/K/V/O x 5) run as fp8e4m3 DoubleRow matmuls
(256-deep contraction per instruction at 0.5 cycles/row): weights are
quantized host-side with per-matrix power-of-2 scales, activations are
quantized on device with a fixed x16 (x32 for attention outputs) scale,
and the combined dequant factor folds into each projection's PSUM-drain
epilogue. The FFN and the attention core (scores/softmax/numerators) stay
bf16/f32: fp8 there would push the final error past the tolerance.
"""

import sys

sys.path.insert(0, "/opt/trn_rl_repo")

import numpy as np
import ml_dtypes

import concourse.bass as bass
import concourse.tile as tile
from concourse import bacc, mybir
from concourse.bass_utils import run_bass_kernel_spmd
from concourse.masks import make_identity
from contextlib import ExitStack

F32 = mybir.dt.float32
BF16 = mybir.dt.bfloat16
F8 = mybir.dt.float8e4
AF = mybir.ActivationFunctionType
ALU = mybir.AluOpType
DR = mybir.MatmulPerfMode.DoubleRow
BF = ml_dtypes.bfloat16
F8NP = ml_dtypes.float8_e4m3

P = 128
D = 1024
KT = 8           # number of 128-partition tiles covering D
NJK = 4          # number of 256-deep k-pairs covering D (DoubleRow)
T = 1024         # tokens per core (2 batches x 512)
CH = 512         # free-dim chunk = one local batch worth of tokens
NCH = 2          # chunks (= local batches)
H = 16
DK = 64
DFF = 4096
NJ = DFF // P    # 32
N1 = 512
EPS = 1e-5
NEG = -1e9
SH = 16.0        # fp8 scale for layernormed activations and word features
SA = 32.0        # fp8 scale for attention outputs (pre-O-projection)

_CACHE = {}


def _build(dqw):
    """dqw[ai][j]: dequant factor folded into the PSUM-drain epilogue of
    attention projection (ai, j) — 1/(act_scale * weight_scale)."""
    nc = bacc.Bacc("TRN2", target_bir_lowering=False, debug=False, num_devices=8)

    xT = nc.dram_tensor("xT", [NCH, D, N1], F32, kind="ExternalInput")
    maskTd = nc.dram_tensor("maskTd", [NCH, 4, P, P], BF16, kind="ExternalInput")
    cptT = nc.dram_tensor("cptT", [NCH, D, 25], F8, kind="ExternalInput")
    senT = nc.dram_tensor("senT", [NCH, D, 25], F8, kind="ExternalInput")
    regT = nc.dram_tensor("regT", [NCH, D, 196], F8, kind="ExternalInput")
    spaT = nc.dram_tensor("spaT", [NCH, D, 196], F8, kind="ExternalInput")
    attW = nc.dram_tensor("attW", [5, 4, D, D], F8, kind="ExternalInput")
    ffnW1 = nc.dram_tensor("ffnW1", [D, DFF], BF16, kind="ExternalInput")
    ffnW2 = nc.dram_tensor("ffnW2", [DFF, D], BF16, kind="ExternalInput")
    bqd = nc.dram_tensor("bq", [5, KT, P], F32, kind="ExternalInput")
    boed = nc.dram_tensor("boe", [5, KT, P], F32, kind="ExternalInput")
    b1d = nc.dram_tensor("b1", [NJ, P], F32, kind="ExternalInput")
    browd = nc.dram_tensor("brow", [1, 1, D], BF16, kind="ExternalInput")  # b2
    outT = nc.dram_tensor("outT", [NCH, D, N1], F32, kind="ExternalOutput")

    words = {"cpt": (cptT, 25), "sen": (senT, 25), "reg": (regT, 196), "spa": (spaT, 196)}

    with tile.TileContext(nc) as tc, ExitStack() as ctx:
        const = ctx.enter_context(tc.tile_pool(name="const", bufs=1))
        trunk_pool = ctx.enter_context(tc.tile_pool(name="trunk", bufs=1))
        spool = ctx.enter_context(tc.tile_pool(name="stats", bufs=1))
        tmp = ctx.enter_context(tc.tile_pool(name="tmps", bufs=2))
        wpool = ctx.enter_context(tc.tile_pool(name="weights", bufs=3))
        hpool = ctx.enter_context(tc.tile_pool(name="hq", bufs=1))
        apool = ctx.enter_context(tc.tile_pool(name="attn", bufs=1))
        psum = ctx.enter_context(tc.tile_pool(name="psum", bufs=1, space="PSUM"))
        prpool = ctx.enter_context(tc.tile_pool(name="probs", bufs=7))
        rcpool = ctx.enter_context(tc.tile_pool(name="recip", bufs=4))
        nmopool = ctx.enter_context(tc.tile_pool(name="nmo", bufs=4))
        mkpool = ctx.enter_context(tc.tile_pool(name="mk", bufs=1))
        w1pool = ctx.enter_context(tc.tile_pool(name="w1p", bufs=4))
        w2pool = ctx.enter_context(tc.tile_pool(name="w2p", bufs=3))
        outsb = ctx.enter_context(tc.tile_pool(name="outsb", bufs=2))

        _PBUFS = {"pA": 4, "pB": 2, "pC": 2}
        _pcnt = [0]

        def ptile(tag, shape=None, dtype=F32):
            # 8 PSUM banks shared by the whole kernel: 4+2+2 slots
            _pcnt[0] += 1
            return psum.tile(shape or [P, CH], dtype, name=f"{tag}_{_pcnt[0]}",
                             tag=tag, bufs=_PBUFS[tag])

        def dma(out, in_):
            nc.sync.dma_start(out=out, in_=in_)

        ident = const.tile([P, P], BF16, name="ident", tag="ident")
        make_identity(nc, ident)
        ones_col = const.tile([P, 1], BF16, name="ones_col", tag="ones_col")
        nc.vector.memset(ones_col, 1.0)
        ones_row = const.tile([1, P], BF16, name="ones_row", tag="ones_row")
        nc.vector.memset(ones_row, 1.0)
        ones_cn = const.tile([1, CH], BF16, name="ones_cn", tag="ones_cn")
        nc.vector.memset(ones_cn, 1.0)
        eps_sb = const.tile([P, 1], F32, name="eps_sb", tag="eps_sb")
        nc.vector.memset(eps_sb, EPS)
        brow = const.tile([1, 1, D], BF16, name="brow", tag="brow")
        dma(brow[:], browd.ap())

        def load_bias(dram, row, ntiles, tag):
            t = const.tile([P, ntiles], F32, tag=tag)
            src = dram.ap()[row] if row is not None else dram.ap()
            dma(t[:, :], src.rearrange("j p -> p j"))
            return t

        bq_sb = [load_bias(bqd, i, KT, f"bq{i}") for i in range(5)]
        boe_sb = [load_bias(boed, i, KT, f"boe{i}") for i in range(5)]
        b1_sb = load_bias(b1d, None, NJ, "b1")

        # trunk = captions^T, f32, updated in place through the layer
        trunk = []
        for k in range(KT):
            t = trunk_pool.tile([P, T], F32, name=f"c{k}", tag=f"c{k}")
            dma(t.rearrange("p (b n) -> p b n", b=NCH),
                xT.ap()[:, k * P:(k + 1) * P, :].rearrange("b p n -> p b n"))
            trunk.append(t)

        # ---------- layernorm (feature-major); gamma/beta folded into weights ----------
        def ln_stats(scale=1.0):
            """Returns (u_sb, v_sb): [128, T] tiles with scale*rstd and
            -scale*mean*rstd broadcast along partitions: (x*u + v) is the
            normalized input times `scale`."""
            u_sb = spool.tile([P, T], BF16, name="u_sb", tag="u_sb")
            v_sb = spool.tile([P, T], BF16, name="v_sb", tag="v_sb")
            if True:
                for ch in range(NCH):
                    cs = slice(ch * CH, (ch + 1) * CH)
                    s1 = ptile("pC", [1, CH])
                    s2 = ptile("pC", [1, CH])
                    for k in range(KT):
                        xc = tmp.tile([P, CH], BF16, name="xc", tag="xc")
                        nc.vector.tensor_copy(xc[:], trunk[k][:, cs])
                        sq = tmp.tile([P, CH], BF16, name="sq", tag="sq")
                        nc.vector.tensor_mul(sq[:], xc[:], xc[:])
                        nc.tensor.matmul(s1[:], lhsT=ones_col[:], rhs=xc[:],
                                         start=(k == 0), stop=(k == KT - 1))
                        nc.tensor.matmul(s2[:], lhsT=ones_col[:], rhs=sq[:],
                                         start=(k == 0), stop=(k == KT - 1))
                    # rs = 1/sqrt((s2 - s1^2/D)/D + eps); u = sc*rs; v = -sc*(s1/D)*rs
                    s1c = tmp.tile([1, CH], F32, name="rs1", tag="rA", bufs=2)
                    nc.vector.tensor_copy(s1c[:], s1[:])
                    q = tmp.tile([1, CH], F32, name="rq", tag="rB", bufs=4)
                    nc.vector.tensor_mul(q[:], s1c[:], s1c[:])
                    v2 = tmp.tile([1, CH], F32, name="rv2", tag="rB", bufs=4)
                    nc.vector.tensor_scalar(out=v2[:], in0=q[:], scalar1=-1.0 / D,
                                            scalar2=None, op0=ALU.mult)
                    nc.vector.tensor_add(v2[:], v2[:], s2[:])
                    sd = tmp.tile([1, CH], F32, name="rsd", tag="rB", bufs=4)
                    nc.scalar.activation(sd[:], v2[:], AF.Sqrt, bias=eps_sb[0:1, :], scale=1.0 / D)
                    rs = tmp.tile([1, CH], F32, name="rrs", tag="rB", bufs=4)
                    nc.vector.reciprocal(rs[:], sd[:])
                    u_bf = tmp.tile([1, CH], BF16, name="ru", tag="rC", bufs=2)
                    nc.vector.tensor_scalar(out=u_bf[:], in0=rs[:], scalar1=scale,
                                            scalar2=None, op0=ALU.mult)
                    w = tmp.tile([1, CH], F32, name="rw", tag="rB", bufs=4)
                    nc.vector.tensor_mul(w[:], s1c[:], rs[:])
                    v_bf = tmp.tile([1, CH], BF16, name="rvb", tag="rC", bufs=2)
                    nc.vector.tensor_scalar(out=v_bf[:], in0=w[:], scalar1=-scale / D,
                                            scalar2=None, op0=ALU.mult)
                    ub = ptile("pB")
                    nc.tensor.matmul(ub[:], lhsT=ones_row[:], rhs=u_bf[:], start=True, stop=True)
                    nc.vector.tensor_copy(u_sb[:, cs], ub[:])
                    vb = ptile("pB")
                    nc.tensor.matmul(vb[:], lhsT=ones_row[:], rhs=v_bf[:], start=True, stop=True)
                    nc.vector.tensor_copy(v_sb[:, cs], vb[:])
            return u_sb, v_sb

        def ln_nrm_f8(u_sb, v_sb):
            """nrm pairs: 4 tiles [P, 2, T] fp8 with (trunk*u + v)."""
            hs = []
            for jk in range(NJK):
                h = hpool.tile([P, 2, T], F8, name=f"hp{jk}", tag=f"hp{jk}")
                for e in range(2):
                    k = 2 * jk + e
                    for ch in range(NCH):
                        cs = slice(ch * CH, (ch + 1) * CH)
                        t1 = tmp.tile([P, CH], F32, name="lnt", tag="lnt")
                        nc.vector.tensor_mul(t1[:], trunk[k][:, cs], u_sb[:, cs])
                        nc.vector.tensor_add(h[:, e, cs], t1[:], v_sb[:, cs])
                hs.append(h)
            return hs

        def ln_nrm_bf(u_sb, v_sb):
            """nrm[k] = trunk[k]*u + v -> bf16 tiles (tag h5_{k})."""
            hs = []
            for k in range(KT):
                h = hpool.tile([P, T], BF16, name=f"h5_{k}", tag=f"h5_{k}")
                for ch in range(NCH):
                    cs = slice(ch * CH, (ch + 1) * CH)
                    t1 = tmp.tile([P, CH], F32, name="lnt", tag="lnt")
                    nc.vector.tensor_mul(t1[:], trunk[k][:, cs], u_sb[:, cs])
                    nc.vector.tensor_add(h[:, cs], t1[:], v_sb[:, cs])
                hs.append(h)
            return hs

        # ---------- weight streaming: quarter-matrices [128, KT, 256] fp8 ----------
        QW = 256

        def load_attw_q(ai, j, qtr):
            t = wpool.tile([P, KT, QW], F8, name="W", tag="W")
            dma(t[:], attW.ap()[ai, j, :, qtr * QW:(qtr + 1) * QW]
                .rearrange("(k p) c -> p k c", p=P))
            return t

        def proj_dr(ai, j, hp, emit):
            """DoubleRow projection: for each m-tile and token chunk,
            accumulate the K=1024 contraction as 4 k-pair matmuls per
            256-col half, then emit(m, ch, psum[P, CH])."""
            for qtr in range(4):
                w = load_attw_q(ai, j, qtr)
                for mh in range(2):
                    m = qtr * 2 + mh
                    for ch in range(NCH):
                        ps = ptile("pA")
                        for nh in range(2):
                            c0 = ch * CH + nh * QW
                            for jk in range(NJK):
                                nc.tensor.matmul(
                                    ps[:, nh * QW:(nh + 1) * QW],
                                    lhsT=w[:, 2 * jk:2 * jk + 2, mh * P:(mh + 1) * P],
                                    rhs=hp[jk][:, :, c0:c0 + QW],
                                    start=(jk == 0), stop=(jk == NJK - 1),
                                    perf_mode=DR)
                        emit(m, ch, ps)

        QT = [hpool.tile([P, T], BF16, name=f"q{m}", tag=f"q{m}") for m in range(KT)]
        ATp = [apool.tile([P, 2, T], F8, name=f"aTp{g}", tag=f"aTp{g}") for g in range(NJK)]

        def attn_core_pair(b, hp, probs2, plens, qoffs, VW):
            """numerators + normalize for heads (2hp, 2hp+1), one shared
            transpose of the [128, 128] pair block into ATp[hp//1...].

            probs2[e][i] holds q columns starting at global offset qoffs[i]."""
            for qt in range(4):
                nmo2 = nmopool.tile([P, 2 * DK], BF16, name="nmo2", tag="nmo2")
                for e in range(2):
                    h = 2 * hp + e
                    nm = ptile("pA", [P, 65])
                    kts = [i for i in range(len(probs2[e])) if qoffs[i] <= qt * P]
                    for n, i in enumerate(kts):
                        sz = plens[i]
                        c0 = qt * P - qoffs[i]
                        nc.tensor.matmul(nm[:], lhsT=probs2[e][i][:sz, c0:c0 + P],
                                         rhs=VW[i][:sz, 65 * h:65 * h + 65],
                                         start=(n == 0), stop=(n == len(kts) - 1))
                    rc = rcpool.tile([P, 1], F32, name="rc", tag="rc")
                    nc.vector.reciprocal(rc[:], nm[:, 64:65])
                    nc.any.tensor_scalar(out=nmo2[:, e * DK:(e + 1) * DK], in0=nm[:, 0:DK],
                                         scalar1=rc[:], scalar2=None, op0=ALU.mult)
                tp = ptile("pB", [P, P], BF16)
                nc.tensor.transpose(tp[:], nmo2[:], ident[:])
                nc.any.tensor_scalar(
                    out=ATp[hp // 2][:, hp % 2, b * CH + qt * P:b * CH + (qt + 1) * P],
                    in0=tp[:], scalar1=SA, scalar2=None, op0=ALU.mult)

        def o_proj(ai, residual, dst):
            """residual: trunk += dq*psum + boe; else dst[m] = dq*psum + boe."""
            dq = dqw[ai][3]

            def emit(m, ch, ps):
                cs = slice(ch * CH, (ch + 1) * CH)
                if residual:
                    t = tmp.tile([P, CH], BF16, name="ot8", tag="ot8")
                    nc.scalar.activation(t[:], ps[:], AF.Identity,
                                         bias=boe_sb[ai][:, m:m + 1], scale=dq)
                    nc.vector.tensor_add(trunk[m][:, cs], trunk[m][:, cs], t[:])
                else:
                    nc.scalar.activation(dst[m][:, cs], ps[:], AF.Identity,
                                         bias=boe_sb[ai][:, m:m + 1], scale=dq)

            proj_dr(ai, 3, ATp, emit)

        # ==================== LN0 + self-attention ====================
        u0, v0 = ln_stats(SH)
        h0 = ln_nrm_f8(u0, v0)

        KTl = [apool.tile([P, T], BF16, name=f"kt{m}", tag=f"kt{m}") for m in range(KT)]
        VA = [apool.tile([P, H * 65], BF16, name=f"va{g}", tag=f"va{g}") for g in range(KT)]
        if True:
            def emit_q(ai):
                dq = dqw[ai][0]

                def emit(m, ch, ps):
                    nc.scalar.activation(QT[m][:, ch * CH:(ch + 1) * CH], ps[:],
                                         AF.Identity, bias=bq_sb[ai][:, m:m + 1],
                                         scale=dq)
                return emit

            def emit_k_self(m, ch, ps):
                nc.any.tensor_scalar(out=KTl[m][:, ch * CH:(ch + 1) * CH], in0=ps[:],
                                     scalar1=dqw[0][1], scalar2=None, op0=ALU.mult)

            proj_dr(0, 0, h0, emit_q(0))
            proj_dr(0, 1, h0, emit_k_self)
            for g in range(KT):
                nc.vector.memset(VA[g].rearrange("p (h c) -> p h c", c=65)[:, :, 64:65], 1.0)
            for qtr in range(4):
                wv = load_attw_q(0, 2, qtr)
                for g in range(KT):
                    ps = ptile("pB", [P, QW])
                    for jk in range(NJK):
                        nc.tensor.matmul(ps[:],
                                         lhsT=h0[jk][:, :, g * P:(g + 1) * P],
                                         rhs=wv[:, 2 * jk:2 * jk + 2, :],
                                         start=(jk == 0), stop=(jk == NJK - 1),
                                         perf_mode=DR)
                    dst = VA[g].rearrange("p (h c) -> p h c", c=65)[:, qtr * 4:(qtr + 1) * 4, 0:64]
                    nc.any.tensor_scalar(out=dst, in0=ps.rearrange("p (h c) -> p h c", c=64),
                                         scalar1=dqw[0][2], scalar2=None, op0=ALU.mult)

        if True:
            for b in range(NCH):
                mkt = mkpool.tile([P, 4, P], BF16, name="mkt", tag="mkt")
                dma(mkt[:], maskTd.ap()[b].rearrange("t p c -> p t c"))
                mk = [mkt[:, kt] for kt in range(4)]
                for hp in range(H // 2):
                    mt = hp
                    probs2 = []
                    for e in range(2):
                        po = e * 64
                        probs = []
                        for kt in range(4):
                            nq = CH - kt * P
                            q0 = kt * P
                            ps = ptile("pC")
                            nc.tensor.matmul(
                                ps[:, :nq],
                                lhsT=KTl[mt][po:po + 64, b * CH + kt * P:b * CH + (kt + 1) * P],
                                rhs=QT[mt][po:po + 64, b * CH + q0:(b + 1) * CH],
                                start=True, stop=False)
                            # additive mask via PE: ps[:,0:P] += mkq.T @ I
                            nc.tensor.matmul(ps[:, 0:P], lhsT=mk[kt][:], rhs=ident[:],
                                             start=False, stop=True,
                                             skip_group_check=True)
                            pr = prpool.tile([P, CH], BF16, name="pr", tag="pr")
                            nc.scalar.activation(pr[:, :nq], ps[:, :nq], AF.Exp)
                            probs.append(pr)
                        probs2.append(probs)
                    qoffs = [kt * P for kt in range(4)]
                    attn_core_pair(b, hp, probs2, [P] * 4, qoffs, VA[4 * b:4 * b + 4])
        o_proj(0, True, None)

        # ============ shared LN stats + normalized c; cross attentions ============
        uc, vc = ln_stats(SH)
        nrm_c = ln_nrm_f8(uc, vc)

        def cross_attention(ai, name, dst):
            w_dram, L = words[name]
            nkt = (L + P - 1) // P
            lsz = [min(P, L - kt * P) for kt in range(nkt)]
            wt_all = spool.tile([P, KT, NCH, L], F8, name="wt_all", tag="wt_all")
            for b in range(NCH):
                dma(wt_all[:, :, b, :], w_dram.ap()[b].rearrange("(k p) l -> p k l", p=P))
            KW = [spool.tile([P, NCH, L], BF16, name=f"kw{m}", tag=f"kw{m}") for m in range(KT)]
            VW = [spool.tile([P, H * 65], BF16, name=f"vaw{b}_{kt}", tag=f"vaw{b}_{kt}")
                  for b in range(NCH) for kt in range(nkt)]
            if True:
                proj_dr(ai, 0, nrm_c, emit_q(ai))
                for qtr in range(4):
                    wk = load_attw_q(ai, 1, qtr)
                    for mh in range(2):
                        m = qtr * 2 + mh
                        for b in range(NCH):
                            ps = ptile("pC", [P, L])
                            for jk in range(NJK):
                                nc.tensor.matmul(
                                    ps[:],
                                    lhsT=wk[:, 2 * jk:2 * jk + 2, mh * P:(mh + 1) * P],
                                    rhs=wt_all[:, 2 * jk:2 * jk + 2, b, :],
                                    start=(jk == 0), stop=(jk == NJK - 1),
                                    perf_mode=DR)
                            nc.any.tensor_scalar(out=KW[m][:, b, :], in0=ps[:],
                                                 scalar1=dqw[ai][1], scalar2=None,
                                                 op0=ALU.mult)
                for va in VW:
                    nc.vector.memset(va.rearrange("p (h c) -> p h c", c=65)[:, :, 64:65], 1.0)
                for qtr in range(4):
                    wv = load_attw_q(ai, 2, qtr)
                    for b in range(NCH):
                        for kt in range(nkt):
                            sz = lsz[kt]
                            ps = ptile("pB", [P, QW])
                            for jk in range(NJK):
                                nc.tensor.matmul(
                                    ps[:sz, :],
                                    lhsT=wt_all[:, 2 * jk:2 * jk + 2, b, kt * P:kt * P + sz],
                                    rhs=wv[:, 2 * jk:2 * jk + 2, :],
                                    start=(jk == 0), stop=(jk == NJK - 1),
                                    perf_mode=DR)
                            va = VW[b * nkt + kt]
                            dstv = va.rearrange("p (h c) -> p h c", c=65)[:sz, qtr * 4:(qtr + 1) * 4, 0:64]
                            nc.any.tensor_scalar(out=dstv,
                                                 in0=ps[:sz].rearrange("p (h c) -> p h c", c=64),
                                                 scalar1=dqw[ai][2], scalar2=None,
                                                 op0=ALU.mult)
            if True:
                for b in range(NCH):
                    for hp in range(H // 2):
                        mt = hp
                        probs2 = []
                        for e in range(2):
                            po = e * 64
                            probs = []
                            for kt in range(nkt):
                                sz = lsz[kt]
                                ps = ptile("pC")
                                nc.tensor.matmul(ps[:sz, :],
                                                 lhsT=KW[mt][po:po + 64, b, kt * P:kt * P + sz],
                                                 rhs=QT[mt][po:po + 64, b * CH:(b + 1) * CH],
                                                 start=True, stop=True)
                                pr = prpool.tile([P, CH], BF16, name="pr", tag="pr")
                                nc.scalar.activation(pr[:sz, :], ps[:sz, :], AF.Exp)
                                probs.append(pr)
                            probs2.append(probs)
                        attn_core_pair(b, hp, probs2, lsz, [0] * nkt, VW[b * nkt:(b + 1) * nkt])
            o_proj(ai, False, dst)

        def fuse_gate(x1, x2):
            """trunk += 0.5 * softmax-gated combination of x1, x2 (vs pre-gate c).

            The gate-score reads of trunk are emitted before the in-place
            trunk updates below; Tile orders them correctly."""
            if True:
                for ch in range(NCH):
                    cs = slice(ch * CH, (ch + 1) * CH)
                    srow = []
                    for xi in (x1, x2):
                        s = ptile("pB", [1, CH])
                        for k in range(KT):
                            cbf = tmp.tile([P, CH], BF16, name="gcb", tag="gcb")
                            nc.vector.tensor_copy(cbf[:], trunk[k][:, cs])
                            nc.vector.tensor_mul(cbf[:], xi[k][:, cs], cbf[:])
                            nc.tensor.matmul(s[:], lhsT=ones_col[:], rhs=cbf[:],
                                             start=(k == 0), stop=(k == KT - 1))
                        srow.append(s)
                    e0 = tmp.tile([1, CH], F32, name="ge0", tag="rA", bufs=2)
                    nc.scalar.activation(e0[:], srow[0][:], AF.Exp)
                    e1 = tmp.tile([1, CH], F32, name="ge1", tag="rA", bufs=2)
                    nc.scalar.activation(e1[:], srow[1][:], AF.Exp)
                    ss = tmp.tile([1, CH], F32, name="gss", tag="rB", bufs=4)
                    nc.vector.tensor_add(ss[:], e0[:], e1[:])
                    r = tmp.tile([1, CH], F32, name="gr", tag="rB", bufs=4)
                    nc.vector.reciprocal(r[:], ss[:])
                    r2 = tmp.tile([1, CH], F32, name="gr2", tag="rB", bufs=4)
                    nc.vector.tensor_scalar(out=r2[:], in0=r[:], scalar1=0.5,
                                            scalar2=None, op0=ALU.mult)
                    w0 = tmp.tile([1, CH], BF16, name="gw0", tag="rC", bufs=2)
                    nc.vector.tensor_mul(w0[:], e0[:], r2[:])
                    w1 = tmp.tile([1, CH], BF16, name="gw1", tag="rC", bufs=2)
                    nc.vector.tensor_mul(w1[:], e1[:], r2[:])
                    w0b = ptile("pC")
                    nc.tensor.matmul(w0b[:], lhsT=ones_row[:], rhs=w0[:], start=True, stop=True)
                    w1b = ptile("pC")
                    nc.tensor.matmul(w1b[:], lhsT=ones_row[:], rhs=w1[:], start=True, stop=True)
                    for k in range(KT):
                        t1 = tmp.tile([P, CH], BF16, name="gt1", tag="gt1")
                        nc.vector.tensor_mul(t1[:], x1[k][:, cs], w0b[:])
                        nc.vector.tensor_add(trunk[k][:, cs], trunk[k][:, cs], t1[:])
                        t2 = tmp.tile([P, CH], BF16, name="gt2", tag="gt1")
                        nc.vector.tensor_mul(t2[:], x2[k][:, cs], w1b[:])
                        nc.vector.tensor_add(trunk[k][:, cs], trunk[k][:, cs], t2[:])

        cptO = [apool.tile([P, T], BF16, name=f"cptO{m}", tag=f"kt{m}") for m in range(KT)]
        senO = [apool.tile([P, T], BF16, name=f"senO{g}", tag=f"va{g}") for g in range(KT)]
        cross_attention(1, "cpt", cptO)
        cross_attention(2, "sen", senO)
        fuse_gate(cptO, senO)
        regO = [apool.tile([P, T], BF16, name=f"regO{m}", tag=f"kt{m}") for m in range(KT)]
        spaO = [apool.tile([P, T], BF16, name=f"spaO{g}", tag=f"va{g}") for g in range(KT)]
        cross_attention(3, "reg", regO)
        cross_attention(4, "spa", spaO)
        fuse_gate(regO, spaO)

        # ==================== LN5 + FFN (bf16) ====================
        u5, v5 = ln_stats()
        h5 = ln_nrm_bf(u5, v5)

        def mid_tag(j):
            if j < KT:
                return f"kt{j}"
            if j < 2 * KT:
                return f"va{j - KT}"
            if j < 20:
                return f"aTp{j - 16}"
            if j < 24:
                return f"hp{j - 20}"
            return f"q{j - 24}"

        mid = [(apool if j < 20 else hpool).tile([P, T], BF16, name=f"mid{j}",
                                                 tag=mid_tag(j)) for j in range(NJ)]
        if True:
            for j in range(NJ):
                w1t = w1pool.tile([P, KT, P], BF16, name="w1j", tag="w1j")
                dma(w1t[:], ffnW1.ap()[:, j * P:(j + 1) * P]
                    .rearrange("(k p) c -> p k c", p=P))
                for ch in range(NCH):
                    ps = ptile("pA")
                    for k in range(KT):
                        nc.tensor.matmul(ps[:], lhsT=w1t[:, k, :],
                                         rhs=h5[k][:, ch * CH:(ch + 1) * CH],
                                         start=(k == 0), stop=(k == KT - 1))
                    nc.scalar.activation(mid[j][:, ch * CH:(ch + 1) * CH], ps[:],
                                         AF.Relu, bias=b1_sb[:, j:j + 1])
        if True:
            for ch in range(NCH):
                cs = slice(ch * CH, (ch + 1) * CH)
                _lets = ["pA", "pA", "pA", "pA", "pB", "pB", "pC", "pC"]
                pss = [ptile(_lets[m]) for m in range(KT)]
                for m in range(KT):
                    nc.tensor.matmul(pss[m][:], lhsT=brow[0:1, 0, m * P:(m + 1) * P],
                                     rhs=ones_cn[:], start=True, stop=False)
                for j in range(NJ):
                    w2t = w2pool.tile([P, D], BF16, name="w2j", tag="w2j")
                    dma(w2t[:], ffnW2.ap()[j * P:(j + 1) * P, :])
                    for m in range(KT):
                        nc.tensor.matmul(pss[m][:], lhsT=w2t[:, m * P:(m + 1) * P],
                                         rhs=mid[j][:, cs],
                                         start=False, stop=(j == NJ - 1))
                for m in range(KT):
                    ot = outsb.tile([P, CH], F32, name="ot", tag="ot")
                    nc.vector.tensor_add(ot[:], trunk[m][:, cs], pss[m][:])
                    dma(outT.ap()[ch, m * P:(m + 1) * P, :], ot[:])

    nc.compile()
    return nc


def _check_causal(seq_masks):
    """The block-sparse self-attention path needs: below-diagonal k-blocks
    fully attended, above-diagonal fully masked (any within-diagonal-block
    pattern is handled exactly)."""
    m = np.asarray(seq_masks)
    for b in range(m.shape[0]):
        for qt in range(4):
            for kt in range(4):
                blk = m[b, qt * P:(qt + 1) * P, kt * P:(kt + 1) * P]
                if kt < qt and not (blk == 1).all():
                    return False
                if kt > qt and not (blk == 0).all():
                    return False
    return True


def _wq8(W, target=96.0):
    """power-of-2 scale quantization to fp8 e4m3 (max finite 240)."""
    mx = float(np.abs(W).max())
    s = float(2.0 ** np.floor(np.log2(target / mx))) if mx > 0 else 1.0
    return (W * s).astype(F8NP), s


def _host_prep(inputs):
    captions = np.asarray(inputs["captions"], np.float32)
    seq_masks = np.asarray(inputs["seq_masks"])
    att_W = np.asarray(inputs["att_W"], np.float32)
    att_b = np.asarray(inputs["att_b"], np.float32)
    ln_g = np.asarray(inputs["ln_g"], np.float32)
    ln_b = np.asarray(inputs["ln_b"], np.float32)
    ffn_W1 = np.asarray(inputs["ffn_W1"], np.float32)
    ffn_W2 = np.asarray(inputs["ffn_W2"], np.float32)
    ffn_b1 = np.asarray(inputs["ffn_b1"], np.float32)
    ffn_b2 = np.asarray(inputs["ffn_b2"], np.float32)

    # Fold LN gamma/beta of the query-side layernorm and 1/sqrt(dk) into the
    # projections. Query-LN site for attention i: site 0 for i=0, site i for
    # cross. K bias drops (softmax shift invariance); V bias folds through O.
    Wq = np.empty_like(att_W[:, 0])
    Wk = att_W[:, 1].copy()
    Wv = att_W[:, 2].copy()
    Wo = att_W[:, 3]
    bq = np.empty_like(att_b[:, 0])
    boe = np.empty_like(att_b[:, 3])
    for i in range(5):
        s = 0 if i == 0 else i
        g, b = ln_g[s], ln_b[s]
        Wq[i] = 0.125 * (g[:, None] * att_W[i, 0])
        bq[i] = 0.125 * (b @ att_W[i, 0] + att_b[i, 0])
        if i == 0:
            # self-attention: K and V also consume the normalized input
            Wk[0] = g[:, None] * att_W[0, 1]
            Wv[0] = g[:, None] * att_W[0, 2]
            bV = b @ att_W[0, 2] + att_b[0, 2]
        else:
            bV = att_b[i, 2]
        boe[i] = bV @ att_W[i, 3] + att_b[i, 3]

    # fp8 quantization of the four projections per attention; dequant factors
    # combine the weight scale with the on-device activation scale.
    attW8 = np.empty((5, 4, D, D), F8NP)
    dqw = []
    for i in range(5):
        row = []
        for j, W in enumerate((Wq[i], Wk[i], Wv[i], Wo[i])):
            attW8[i, j], sw = _wq8(W)
            act_s = SA if j == 3 else SH
            row.append(1.0 / (act_s * sw))
        dqw.append(tuple(row))
    dqw = tuple(dqw)

    g5, b5 = ln_g[5], ln_b[5]
    W1 = g5[:, None] * ffn_W1
    b1 = b5 @ ffn_W1 + ffn_b1
    brow = ffn_b2[None, None].astype(BF)

    xT = np.ascontiguousarray(captions.transpose(0, 2, 1))

    def wT8(name):
        a = np.asarray(inputs[name], np.float32) * SH
        return np.ascontiguousarray(a.transpose(0, 2, 1)).astype(F8NP)

    prep = dict(
        xT=xT,
        cptT=wT8("cpt_words"), senT=wT8("senti_words"),
        regT=wT8("region_feats"), spaT=wT8("spatial_feats"),
        attW=attW8,
        ffnW1=W1.astype(BF),
        ffnW2=ffn_W2.astype(BF),
        bq=np.ascontiguousarray(bq.reshape(5, KT, P)),
        boe=np.ascontiguousarray(boe.reshape(5, KT, P)),
        b1=np.ascontiguousarray(b1.reshape(NJ, P)),
        brow=brow,
    )
    mTd = np.zeros((seq_masks.shape[0], 4, P, P), np.float32)
    for kt in range(4):
        blk = seq_masks[:, kt * P:(kt + 1) * P, kt * P:(kt + 1) * P]  # [B, q, k]
        mTd[:, kt] = np.where(blk == 0, np.float32(NEG), 0.0)  # q-major lhsT
    prep["maskTd"] = mTd.astype(BF)
    return prep, dqw


def _numpy_reference(inputs):
    """Fallback for masks outside the block-causal structure the device
    kernel assumes. Bit-accurate fp32 numpy implementation."""
    f = lambda k: np.asarray(inputs[k], np.float32)
    att_W, att_b = f("att_W"), f("att_b")
    ln_g, ln_b = f("ln_g"), f("ln_b")
    mask = np.asarray(inputs["seq_masks"])

    def ln(x, g, b):
        m = x.mean(-1, keepdims=True)
        v = ((x - m) ** 2).mean(-1, keepdims=True)
        return (x - m) / np.sqrt(v + EPS) * g + b

    def mha(q_in, k_in, v_in, W, b, msk=None):
        B_, N = q_in.shape[0], q_in.shape[1]
        def proj(x, i):
            y = x @ W[i] + b[i]
            return y.reshape(x.shape[0], -1, H, DK).transpose(0, 2, 1, 3)
        q, k, v = proj(q_in, 0), proj(k_in, 1), proj(v_in, 2)
        s = (q @ k.transpose(0, 1, 3, 2)) / np.sqrt(DK)
        if msk is not None:
            s = np.where(msk[:, None] == 0, -np.inf, s)
        s = s - s.max(-1, keepdims=True)
        a = np.exp(s)
        a /= a.sum(-1, keepdims=True)
        x = (a @ v).transpose(0, 2, 1, 3).reshape(B_, N, H * DK)
        return x @ W[3] + b[3]

    def gate(x, f1, f2):
        s = np.stack([(f1 * x).sum(-1), (f2 * x).sum(-1)], -1)
        s = s - s.max(-1, keepdims=True)
        w = np.exp(s)
        w /= w.sum(-1, keepdims=True)
        return w[..., 0:1] * f1 + w[..., 1:2] * f2

    c = f("captions")
    h = ln(c, ln_g[0], ln_b[0])
    c = c + mha(h, h, h, att_W[0], att_b[0], mask)
    cpt = mha(ln(c, ln_g[1], ln_b[1]), f("cpt_words"), f("cpt_words"), att_W[1], att_b[1])
    sen = mha(ln(c, ln_g[2], ln_b[2]), f("senti_words"), f("senti_words"), att_W[2], att_b[2])
    sem = gate(c, cpt, sen)
    reg = mha(ln(c, ln_g[3], ln_b[3]), f("region_feats"), f("region_feats"), att_W[3], att_b[3])
    spa = mha(ln(c, ln_g[4], ln_b[4]), f("spatial_feats"), f("spatial_feats"), att_W[4], att_b[4])
    vis = gate(c, reg, spa)
    fuse = c + (sem + vis) * 0.5
    hh = ln(fuse, ln_g[5], ln_b[5])
    return fuse + np.maximum(hh @ f("ffn_W1") + f("ffn_b1"), 0) @ f("ffn_W2") + f("ffn_b2")


def kernel(**inputs) -> np.ndarray:
    if not _check_causal(inputs["seq_masks"]):
        return _numpy_reference(inputs).astype(np.float32)
    prep, dqw = _host_prep(inputs)
    if dqw not in _CACHE:
        _CACHE.clear()
        _CACHE[dqw] = _build(dqw)
    nc = _CACHE[dqw]
    B = inputs["captions"].shape[0]
    n_cores = 8
    bl = B // n_cores
    shared_keys = ("attW", "ffnW1", "ffnW2", "bq", "boe", "b1", "brow")
    per_core_keys = ["xT", "cptT", "senT", "regT", "spaT", "maskTd"]
    in_maps = []
    for i in range(n_cores):
        s = slice(i * bl, (i + 1) * bl)
        m = {k: prep[k] for k in shared_keys}
        for k in per_core_keys:
            m[k] = prep[k][s]
        in_maps.append(m)
    res = run_bass_kernel_spmd(nc, in_maps, list(range(n_cores)))
    out = np.empty((B, N1, D), np.float32)
    for i in range(n_cores):
        out[i * bl:(i + 1) * bl] = res.results[i]["outT"].transpose(0, 2, 1)
    return out
